# revision 1
# baseline (speedup 1.0000x reference)
"""Trainium2 Bass kernel for a separable 3D Haar DWT (nn_DWT3D).

Problem: x [2, 128, 128, 128, 4] fp32, A [128, 128] (orthonormal Haar
analysis filter bank, 2-tap stride-2). Output: subband concat
[2, 64, 64, 64, 32].

Strategy (8 NeuronCores):
- Data-parallel over (batch, channel): B*C = 8 independent [128,128,128]
  volume transforms, one per core. Host deinterleaves channels (numpy
  transpose) so each core's volume is contiguous; host reassembles the
  subband concat at the end.
- Per core, three 1D transforms along i/j/k. A is a 2-tap Haar butterfly
  (L = (x0+x1)/sqrt2, H = (x1-x0)/sqrt2), so:
    * i-axis (SBUF partition axis): one dense PE matmul pass with
      lhsT = (0.25*sqrt2... actually h^2)*A^T, folding the j/k pass
      scales into the PE weights.
    * j-axis and k-axis (free axis): DVE add/sub butterflies (unscaled).
  PSUM->SBUF copies on the scalar engine; everything is streamed in
  1 MB chunks so DMA in/out, PE, ACT and DVE overlap.
"""

import numpy as np

_N = 128
_CORES = 8
_JB = 16                 # j values per chunk
_NCHUNK = _N // _JB      # 8 chunks
_CW = _JB * _N           # 2048 columns per chunk

_cache = {}


def _build():
    """Build + compile the single-core SPMD Bass program."""
    import concourse.mybir as mybir
    from concourse import bacc
    from concourse.tile import TileContext

    nc = bacc.Bacc("TRN2", target_bir_lowering=False, debug=False,
                   num_devices=_CORES)
    v = nc.dram_tensor("v", [_N, _N * _N], mybir.dt.float32,
                       kind="ExternalInput")
    w = nc.dram_tensor("w", [_N, _N], mybir.dt.float32,
                       kind="ExternalInput")
    y = nc.dram_tensor("y", [_N, _N * _N], mybir.dt.float32,
                       kind="ExternalOutput")
    f32 = mybir.dt.float32

    with TileContext(nc) as tc:
        with (
            tc.tile_pool(name="wpool", bufs=1) as wpool,
            tc.tile_pool(name="vin", bufs=3) as vpool,
            tc.tile_pool(name="y1", bufs=2) as y1pool,
            tc.tile_pool(name="y2", bufs=2) as y2pool,
            tc.tile_pool(name="y3", bufs=3) as y3pool,
            tc.tile_pool(name="ps", bufs=8, space="PSUM") as pspool,
        ):
            wt = wpool.tile([_N, _N], f32)
            nc.sync.dma_start(out=wt[:], in_=w[:])
            for cb in range(_NCHUNK):
                vin = vpool.tile([_N, _CW], f32)
                nc.sync.dma_start(out=vin[:], in_=v[:, cb * _CW:(cb + 1) * _CW])

                # i-axis transform: out[a, col] = sum_i w[i, a] * vin[i, col]
                y1 = y1pool.tile([_N, _CW], f32)
                for m in range(_CW // 512):
                    ps = pspool.tile([_N, 512], f32)
                    nc.tensor.matmul(ps[:], wt[:], vin[:, m * 512:(m + 1) * 512],
                                     start=True, stop=True)
                    nc.scalar.copy(out=y1[:, m * 512:(m + 1) * 512], in_=ps[:])

                # j-axis butterfly (pairs along j, blocks of 128 columns)
                y1r = y1[:].rearrange("p (j k) -> p j k", k=_N)
                y2 = y2pool.tile([_N, _CW], f32)
                y2r = y2[:].rearrange("p (t k) -> p t k", k=_N)
                half = _JB // 2
                nc.vector.tensor_add(out=y2r[:, 0:half, :],
                                     in0=y1r[:, 0:_JB:2, :],
                                     in1=y1r[:, 1:_JB:2, :])
                nc.vector.tensor_sub(out=y2r[:, half:_JB, :],
                                     in0=y1r[:, 1:_JB:2, :],
                                     in1=y1r[:, 0:_JB:2, :])

                # k-axis butterfly (pairs along k, stride-2 in free dim)
                y3 = y3pool.tile([_N, _CW], f32)
                y3r = y3[:].rearrange("p (t k) -> p t k", k=_N)
                nc.vector.tensor_add(out=y3r[:, :, 0:64],
                                     in0=y2r[:, :, 0:_N:2],
                                     in1=y2r[:, :, 1:_N:2])
                nc.vector.tensor_sub(out=y3r[:, :, 64:_N],
                                     in0=y2r[:, :, 1:_N:2],
                                     in1=y2r[:, :, 0:_N:2])

                nc.sync.dma_start(out=y[:, cb * _CW:(cb + 1) * _CW], in_=y3[:])

    nc.compile()
    return nc


def _get_nc():
    if "nc" not in _cache:
        _cache["nc"] = _build()
    return _cache["nc"]


def _haar_structure_ok(A):
    """Check A is the expected 2-tap stride-2 filter bank with equal-magnitude
    taps (h, h) lowpass / (-g, g) highpass and h == g, which is what the
    butterfly passes hardcode."""
    if A.shape != (_N, _N):
        return False
    h = A[0, 0]
    if not np.isfinite(h) or abs(h) < 1e-8:
        return False
    expect = np.zeros((_N, _N), dtype=np.float32)
    for i in range(_N // 2):
        expect[i, 2 * i] = h
        expect[i, 2 * i + 1] = h
        expect[_N // 2 + i, 2 * i] = -h
        expect[_N // 2 + i, 2 * i + 1] = h
    return bool(np.allclose(A, expect, rtol=1e-5, atol=1e-7))


def _reference_host(x, A):
    """Generic numpy fallback (slow) for non-Haar A."""
    y = np.einsum("ai,nijkc->najkc", A, x, optimize=True)
    y = np.einsum("bj,najkc->nabkc", A, y, optimize=True)
    y = np.einsum("dk,nabkc->nabdc", A, y, optimize=True)
    return y


def _assemble(y_full, B, C):
    """Slice the transformed volumes y_full [B, C, 128,128,128] into the
    reference's subband concat [B, 64, 64, 64, 8*C]."""
    L, H = slice(0, 64), slice(64, 128)
    bands = [(L, L, L), (H, L, L), (L, H, L), (H, H, L),
             (L, L, H), (H, H, H), (L, H, H), (H, H, H)]
    out = np.empty((B, 64, 64, 64, 8 * C), dtype=np.float32)
    for s, (sa, sb, sd) in enumerate(bands):
        # [B, C, 64, 64, 64] -> [B, 64, 64, 64, C]
        sub = y_full[:, :, sa, sb, sd]
        out[..., s * C:(s + 1) * C] = np.moveaxis(sub, 1, -1)
    return out


def kernel(x, A):
    from concourse.bass_utils import run_bass_kernel_spmd

    x = np.asarray(x, dtype=np.float32)
    A = np.asarray(A, dtype=np.float32)
    B, _, _, _, C = x.shape
    assert (B, C) == (2, 4) and x.shape[1:4] == (_N, _N, _N)

    if not _haar_structure_ok(A):
        y = _reference_host(x, A)
        return _assemble(np.moveaxis(y, -1, 1), B, C)

    h = float(A[0, 0])
    # Fold the j/k butterfly scales (h each) into the PE weights.
    w = np.ascontiguousarray((h * h) * A.T)

    # Shard: volume per (b, c), contiguous [i, j, k].
    xs = np.ascontiguousarray(np.transpose(x, (0, 4, 1, 2, 3)))
    xs = xs.reshape(_CORES, _N, _N * _N)
    in_maps = [{"v": xs[g], "w": w} for g in range(_CORES)]

    nc = _get_nc()
    res = run_bass_kernel_spmd(nc, in_maps, list(range(_CORES)))

    # Unscramble per-core output: y [a, cb, t, kappa] -> vol [a, j', d]
    y_full = np.empty((B, C, _N, _N, _N), dtype=np.float32)
    for g in range(_CORES):
        z = res.results[g]["y"].reshape(_N, _NCHUNK, _JB, _N)
        vol = y_full[g // C, g % C]
        vol[:, 0:64, :] = z[:, :, 0:_JB // 2, :].reshape(_N, 64, _N)
        vol[:, 64:_N, :] = z[:, :, _JB // 2:_JB, :].reshape(_N, 64, _N)
    return _assemble(y_full, B, C)
